# revision 1
# baseline (speedup 1.0000x reference)
"""Trainium2 Bass kernel for GCL contrastive-scoring GNN message passing.

Computation (see the reference):
  h   = x @ W + b                      [N, H]
  q   = sigmoid(h)                     [N, H]
  k_p = normalize(segsum(pw*h))        [Np, H]
  k_n = segsum(pw*q)                   [Np, H]
  att = exp(k_p @ k_p.T / T)           [Np, Np]
  pos = (att * A_P) @ k_n ; neg = att @ k_n
  loss = mean(-log(q.pos[par]) + log(q.neg[par]))

v2: fp8(e4m3) DoubleRow matmuls for encoder / segment sums / attention /
messages; q resident in SBUF; per-window fp8 AllGather overlapping
phase 1; k_p computed as segsum(pw*x) @ W (linearity) so phase 1 reads
PSUM only once per tile (sigmoid); fused multiply-reduce score ops
balanced across DVE and ACT+Pool; phase 3 emitted per message i-half.

Scale conventions (all cancel in the final loss):
  W staged as 8W      (sigmoid applies 1/8; k_p normalization cancels it)
  k_p staged as 16*k_p (exp scale absorbs 1/256)
  msgs staged as msg/512 (log-difference cancels the constant)
"""

import numpy as np
import ml_dtypes

import concourse.bass as bass
import concourse.bacc as bacc
import concourse.mybir as mybir
import concourse.tile as tile
from concourse import bass_utils

F32 = mybir.dt.float32
BF16 = mybir.dt.bfloat16
F8 = mybir.dt.float8e4
AF = mybir.ActivationFunctionType
ALU = mybir.AluOpType
DR = mybir.MatmulPerfMode.DoubleRow

NCORES = 8
NP = 4096          # parents
BAND = 512         # parents per core
NW = 4             # windows (128 parents) per core
D = 512
H = 512
KT = 4             # 128-tiles along D/H
NJ = NP // 128     # 32 j tiles
XW = (2 * KT + 1) * 128   # per-tile xs row: x d-major | seg 1-hot | x node-major
XCH = 4            # xs tiles per DMA chunk (must be even)
EPS = 1e-12
SCL_H = 8.0        # h staged as 8h
SCL_KP = 16.0      # kp staged as 16 kp
SCL_MSG = 1.0 / 512.0
EXP_SCALE = 2.0 / (SCL_KP * SCL_KP)   # exp((16kp_i . 16kp_j) * s) = exp(2 cos)
NF8 = ml_dtypes.float8_e4m3


# ----------------------------------------------------------------- host prep

def prep_inputs(x, node_to_par, p_weight, A_P, W, b):
    x = np.asarray(x, np.float32)
    par = np.asarray(node_to_par).astype(np.int64)
    pw = np.asarray(p_weight, np.float32)
    A_P = np.asarray(A_P, np.float32)
    W = np.asarray(W, np.float32)
    b = np.asarray(b, np.float32)
    N = x.shape[0]

    # group nodes by (core, window); per-window quota uniform, even # tiles
    grp = par // 128                       # [N] in [0, 32)
    order = np.argsort(grp, kind="stable")
    grp_sorted = grp[order]
    bounds = np.searchsorted(grp_sorted, np.arange(NCORES * NW + 1))
    counts = np.diff(bounds)
    QT = int(np.ceil(max(1, counts.max()) / 128))
    QT += QT % 2                           # even, for DoubleRow tile pairs
    Q0 = QT * 128
    T = NW * QT
    n_pad = NW * Q0

    bias_nonzero = bool(np.any(b != 0.0))

    W8 = np.clip(W * SCL_H, -240, 240).astype(NF8)  # [D, H]
    per_core = []
    for c in range(NCORES):
        slots = np.full(n_pad, -1, np.int64)
        for w in range(NW):
            g = c * NW + w
            lo, hi = bounds[g], bounds[g + 1]
            slots[w * Q0: w * Q0 + (hi - lo)] = order[lo:hi]
        valid = slots >= 0
        sc = np.where(valid, slots, 0)

        # xs[t, p, 0:4, n]   = x[node(t,n), k*128+p]   (d-major, encoder lhsT)
        # xs[t, p, 4, n]     = seg one-hot (pw)        (node on partitions)
        # xs[t, p, 5:9, :]   = x[node(t,p), :]         (node-major, kp segsum)
        x_sel = np.where(valid[:, None], x[sc], 0.0).astype(np.float32)
        xs = np.empty((T, 128, 2 * KT + 1, 128), NF8)
        xs[:, :, :KT, :] = x_sel.reshape(T, 128, KT, 128).transpose(0, 3, 2, 1)
        xs[:, :, KT + 1:, :] = x_sel.reshape(T, 128, KT, 128)

        wslot = np.arange(n_pad) // Q0
        cols = np.where(valid, par[sc] - c * BAND - wslot * 128, 0)
        rows = np.arange(n_pad)

        seg = np.zeros((n_pad, 128), np.float32)
        seg[rows[valid], cols[valid]] = pw[sc[valid]]
        xs[:, :, KT, :] = seg.reshape(T, 128, 128).astype(NF8)

        # gather one-hot, [parent, node] per tile, laid out [p, t*128+n]
        gath = np.zeros((n_pad, 128), np.float32)
        gath[rows, cols] = 1.0
        gathA = np.ascontiguousarray(
            gath.reshape(T, 128, 128).transpose(2, 0, 1).reshape(128, T * 128)
        ).astype(NF8)

        maskT = np.ascontiguousarray(valid.reshape(T, 128).T).astype(np.float32)

        # A_P transposed band, laid out [p, jt*512+i]  (p = j within tile)
        A_PT = A_P[c * BAND:(c + 1) * BAND, :].T            # [4096 j, 512 i]
        aptA = np.ascontiguousarray(
            A_PT.reshape(NJ, 128, BAND).transpose(1, 0, 2).reshape(128, NJ * BAND)
        ).astype(NF8)

        per_core.append({
            "xs": np.ascontiguousarray(
                xs.reshape(T, 128, XW).transpose(1, 0, 2)
            ).reshape(128, T * XW),
            "gathA": gathA, "maskT": maskT, "aptA": aptA,
            "W8": np.ascontiguousarray(W8.reshape(KT, 128, H)),
            "ident": np.eye(128).astype(ml_dtypes.bfloat16),
            **({"bvec": (b * SCL_H).reshape(1, H).astype(ml_dtypes.bfloat16)}
               if bias_nonzero else {}),
        })
    meta = {"N": N, "T": T, "QT": QT, "n_pad": n_pad,
            "bias_nonzero": bias_nonzero}
    return per_core, meta


# ------------------------------------------------------------ device program

def build_program(T, QT, bias_nonzero, stage=4, reps=1, no_coll=False):
    nc = bacc.Bacc("TRN2", target_bir_lowering=False, debug=False,
                   num_devices=NCORES)

    i_xs = nc.dram_tensor("xs", [128, T * XW], F8, kind="ExternalInput")
    i_gath = nc.dram_tensor("gathA", [128, T * 128], F8, kind="ExternalInput")
    i_mask = nc.dram_tensor("maskT", [128, T], F32, kind="ExternalInput")
    i_apt = nc.dram_tensor("aptA", [128, NJ * BAND], F8, kind="ExternalInput")
    i_w = nc.dram_tensor("W8", [KT, 128, H], F8, kind="ExternalInput")
    i_id = nc.dram_tensor("ident", [128, 128], BF16, kind="ExternalInput")
    if bias_nonzero:
        i_b = nc.dram_tensor("bvec", [1, H], BF16, kind="ExternalInput")
    o_loss = nc.dram_tensor("loss_part", [128, 1], F32, kind="ExternalOutput")

    with tile.TileContext(nc) as tc:
        with (
            tc.tile_pool(name="const", bufs=1) as constp,
            tc.tile_pool(name="bands", bufs=1) as bandp,
            tc.tile_pool(name="strm", bufs=3) as strm,
            tc.tile_pool(name="strm2", bufs=2) as strm2,
            tc.tile_pool(name="dram", bufs=1, space="DRAM") as dram,
        ):
            # constants / resident tensors
            w_sb = constp.tile([128, KT * H], F8, tag="w")
            for k in range(KT):
                nc.sync.dma_start(w_sb[:, k * H:(k + 1) * H], i_w[k])
            ident = constp.tile([128, 128], BF16, tag="ident")
            nc.sync.dma_start(ident[:], i_id[:])
            gath_sb = constp.tile([128, T * 128], F8, tag="gath")
            nc.sync.dma_start(gath_sb[:], i_gath[:])
            apt_sb = constp.tile([128, NJ * BAND], F8, tag="apt")
            nc.sync.dma_start(apt_sb[:], i_apt[:])
            if bias_nonzero:
                bias_sb = constp.tile([1, H], BF16, tag="bias")
                nc.sync.dma_start(bias_sb[:], i_b[:])
                ones1 = constp.tile([1, 128], BF16, tag="ones1")
                nc.vector.memset(ones1[:], 1.0)
                onecol = constp.tile([128, 2], F8, tag="onecol")
                nc.vector.memset(onecol[:], 1.0)

            # persistent SBUF
            qres = bandp.tile([128, T * H], F8, tag="qres")
            kj_all = bandp.tile([128, NJ * 1024], F8, tag="kj_all")
            kpT_band = bandp.tile([128, KT * BAND], F8, tag="kpT_band")
            kn8_band = bandp.tile([128, NW * H], F8, tag="kn8_band")
            msgs = bandp.tile([128, NW * 1024], F8, tag="msgs")
            sc_pos = bandp.tile([128, T], F32, tag="sc_pos")
            sc_neg = bandp.tile([128, T], F32, tag="sc_neg")

            agin = dram.tile([NW, 128, 1024], F8)

            w_r = w_sb[:].rearrange("p (k h) -> p k h", h=H)
            kpT_r = kpT_band[:].rearrange("p (s i) -> p s i", i=BAND)
            kj_r = kj_all[:].rearrange("p (j x) -> p j x", x=1024)

            for _rep in range(reps):
              # per-rep Shared AllGather outputs (Shared DRAM wants exactly
              # one writing instruction per tensor)
              agout = [dram.tile([NCORES, 128, 1024], F8, addr_space="Shared",
                                 name=f"agout_{_rep}_{w}")
                       for w in range(NW)]
              # ---------------- phase 1: encoder + segment sums ---------------
              ps1 = tc.tile_pool(name=f"ps1_{_rep}", bufs=2, space="PSUM")
              psA = ps1.__enter__()
              for w in range(NW):
                  ps_kpx = psA.tile([128, D], F32, tag="kpx", bufs=1)
                  ps_kn = psA.tile([128, H], F32, tag="kn", bufs=1)
                  if bias_nonzero:
                      ps_spw = psA.tile([128, 1], F32, tag="spw", bufs=1)
                  for ti in range(QT):
                      t = w * QT + ti
                      if ti % XCH == 0:
                          nxc = min(XCH, QT - ti)
                          xch = strm.tile([128, XCH * XW], F8, tag="xch",
                                          bufs=3)
                          nc.sync.dma_start(
                              xch[:, :nxc * XW],
                              i_xs[:, t * XW:(t + nxc) * XW])
                          xch_r = xch[:].rearrange("p (c xw) -> p c xw", xw=XW)
                      ci = ti % XCH

                      ps_h = psA.tile([128, H], F32, tag="h")
                      if bias_nonzero:
                          nc.tensor.matmul(ps_h[:], ones1[:], bias_sb[:],
                                           start=True, stop=False)
                      xt_r = xch[:, ci * XW: ci * XW + KT * 128].rearrange(
                          "p (kk n) -> p kk n", n=128)
                      for kk in range(2):
                          nc.tensor.matmul(
                              ps_h[:], xt_r[:, 2 * kk:2 * kk + 2, :],
                              w_r[:, 2 * kk:2 * kk + 2, :],
                              start=(kk == 0 and not bias_nonzero),
                              stop=(kk == 1), perf_mode=DR)

                      # q8 resident (the only per-tile PSUM read)
                      nc.scalar.activation(qres[:, t * H:(t + 1) * H], ps_h[:],
                                           AF.Sigmoid, scale=1.0 / SCL_H)

                      if ti % 2 == 1:
                          oh_pair = xch_r[:, ci - 1:ci + 1,
                                          KT * 128:KT * 128 + 128]
                          xn_pair = xch_r[:, ci - 1:ci + 1,
                                          (KT + 1) * 128:XW]
                          q_pair = qres[:, (t - 1) * H:(t + 1) * H].rearrange(
                              "p (two h) -> p two h", two=2)
                          nc.tensor.matmul(ps_kpx[:], oh_pair, xn_pair,
                                           start=(ti == 1),
                                           stop=(ti == QT - 1), perf_mode=DR)
                          nc.tensor.matmul(ps_kn[:], oh_pair, q_pair,
                                           start=(ti == 1),
                                           stop=(ti == QT - 1), perf_mode=DR)
                          if bias_nonzero:
                              nc.tensor.matmul(ps_spw[:], oh_pair,
                                               onecol[:].rearrange(
                                                   "p (two c) -> p two c",
                                                   two=2),
                                               start=(ti == 1),
                                               stop=(ti == QT - 1),
                                               perf_mode=DR)

                  # kp = segsum(pw*x) @ W  (+ segsum(pw) * b)
                  kpx_bf = strm2.tile([128, D], BF16, tag="kpx_bf")
                  nc.vector.tensor_copy(kpx_bf[:], ps_kpx[:])
                  kpxT = strm2.tile([128, KT * 128], F8, tag="kpxT")
                  for s in range(KT):
                      ps_t = psA.tile([128, 128], BF16, tag="tr", bufs=2)
                      nc.tensor.transpose(
                          ps_t[:], kpx_bf[:, s * 128:(s + 1) * 128], ident[:])
                      nc.vector.tensor_copy(
                          kpxT[:, s * 128:(s + 1) * 128], ps_t[:])
                  ps_kp = psA.tile([128, H], F32, tag="kp", bufs=1)
                  kpxT_r = kpxT[:].rearrange("p (s q) -> p s q", q=128)
                  for kk in range(2):
                      nc.tensor.matmul(ps_kp[:], kpxT_r[:, 2 * kk:2 * kk + 2],
                                       w_r[:, 2 * kk:2 * kk + 2, :],
                                       start=(kk == 0),
                                       stop=(kk == 1 and not bias_nonzero),
                                       perf_mode=DR)
                  if bias_nonzero:
                      spw8 = strm2.tile([128, 1], BF16, tag="spw8")
                      nc.scalar.copy(spw8[:], ps_spw[:])
                      ps_st = psA.tile([1, 128], BF16, tag="str", bufs=1)
                      nc.tensor.transpose(ps_st[:], spw8[:], ident[:])
                      spwT = strm2.tile([1, 128], BF16, tag="spwT")
                      nc.vector.tensor_copy(spwT[:], ps_st[:])
                      nc.tensor.matmul(ps_kp[:], spwT[:], bias_sb[:],
                                       start=False, stop=True)

                  # normalize k_p rows (scale 8 cancels; stage 16*kp as fp8)
                  kp_raw = strm2.tile([128, H], F32, tag="kp_raw")
                  nc.vector.tensor_copy(kp_raw[:], ps_kp[:])
                  ssq = strm2.tile([128, 1], F32, tag="ssq")
                  tmp = strm2.tile([128, H], F32, tag="nrm_tmp")
                  nc.scalar.activation(tmp[:], kp_raw[:], AF.Square,
                                       accum_out=ssq[:])
                  nrm = strm2.tile([128, 1], F32, tag="nrm")
                  nc.scalar.activation(nrm[:], ssq[:], AF.Sqrt)
                  nc.vector.tensor_scalar_max(nrm[:], nrm[:], EPS)
                  rinv = strm2.tile([128, 1], F32, tag="rinv")
                  nc.vector.reciprocal(rinv[:], nrm[:])
                  rinv16 = strm2.tile([128, 1], F32, tag="rinv16")
                  nc.vector.tensor_scalar_mul(rinv16[:], rinv[:], SCL_KP)
                  kp_bf = strm2.tile([128, H], BF16, tag="kp_bf")
                  nc.scalar.mul(kp_bf[:], kp_raw[:], rinv16[:])
                  nc.scalar.copy(kn8_band[:, w * H:(w + 1) * H], ps_kn[:])

                  # transpose 16*kp (fp8) -> kpT_band [h, i]
                  for s in range(KT):
                      ps_t = psA.tile([128, 128], BF16, tag="tr", bufs=2)
                      nc.tensor.transpose(
                          ps_t[:], kp_bf[:, s * 128:(s + 1) * 128], ident[:])
                      nc.vector.tensor_copy(
                          kpT_band[:, s * BAND + w * 128:
                                   s * BAND + (w + 1) * 128],
                          ps_t[:])

                  if stage >= 2:
                      # stage agin[w] = [kpT(s,q) | kn] and AllGather it
                      nc.sync.dma_start(
                          agin[w, :, 0:KT * 128].rearrange(
                              "p (s q) -> p s q", q=128),
                          kpT_r[:, :, w * 128:(w + 1) * 128])
                      nc.sync.dma_start(agin[w, :, KT * 128:1024],
                                        kn8_band[:, w * H:(w + 1) * H])
                      if no_coll:
                          for bb in range(NCORES):
                              nc.sync.dma_start(agout[w][bb], agin[w])
                      else:
                          nc.gpsimd.collective_compute(
                              "AllGather", ALU.bypass,
                              replica_groups=[list(range(NCORES))],
                              ins=[agin[w].opt()], outs=[agout[w][:].opt()])
              ps1.__exit__(None, None, None)

              if stage >= 3:
                  # -------- phase 2: attention + messages (+ phase 3) --------
                  ps2 = tc.tile_pool(name=f"ps2_{_rep}", bufs=2, space="PSUM")
                  psB = ps2.__enter__()
                  ps3 = tc.tile_pool(name=f"ps3_{_rep}", bufs=1, space="PSUM")
                  psC = ps3.__enter__()
                  for ihalf in range(2):
                      acc = [psB.tile([128, H], F32, tag=f"acc{g}", bufs=1,
                                      name=f"acc{ihalf}{g}")
                             for g in range(4)]
                      for wh in range(2):
                          for bsel in range(NCORES):
                              jg0 = bsel * NW + 2 * wh
                              if ihalf == 0:
                                  for w2 in range(2):
                                      jg = jg0 + w2
                                      nc.sync.dma_start(
                                          kj_all[:, jg * 1024:(jg + 1) * 1024],
                                          agout[2 * wh + w2][bsel])
                              attp = strm.tile([128, 512], F8, tag="attp",
                                               bufs=2)
                              wposp = strm.tile([128, 512], F8, tag="wposp",
                                                bufs=2)
                              for w2 in range(2):
                                  jg = jg0 + w2
                                  ps_att = psB.tile([128, 256], F32, tag="att")
                                  for sp in range(2):
                                      lhsT = kj_all[
                                          :, jg * 1024 + sp * 256:
                                          jg * 1024 + (sp + 1) * 256].rearrange(
                                              "p (two q) -> p two q", two=2)
                                      nc.tensor.matmul(
                                          ps_att[:], lhsT,
                                          kpT_r[:, 2 * sp:2 * sp + 2,
                                                ihalf * 256:(ihalf + 1) * 256],
                                          start=(sp == 0), stop=(sp == 1),
                                          perf_mode=DR)
                                  nc.scalar.activation(
                                      attp[:, w2 * 256:(w2 + 1) * 256],
                                      ps_att[:], AF.Exp, scale=EXP_SCALE)
                                  nc.vector.tensor_mul(
                                      wposp[:, w2 * 256:(w2 + 1) * 256],
                                      attp[:, w2 * 256:(w2 + 1) * 256],
                                      apt_sb[:, jg * BAND + ihalf * 256:
                                             jg * BAND + (ihalf + 1) * 256])
                              attp_r = attp[:].rearrange(
                                  "p (two i) -> p two i", two=2)
                              wposp_r = wposp[:].rearrange(
                                  "p (two i) -> p two i", two=2)
                              kn_pair = kj_r[:, jg0:jg0 + 2, KT * 128:1024]
                              first = (wh == 0 and bsel == 0)
                              last = (wh == 1 and bsel == NCORES - 1)
                              for i2 in range(2):
                                  nc.tensor.matmul(
                                      acc[i2][:],
                                      wposp_r[:, :, i2 * 128:(i2 + 1) * 128],
                                      kn_pair, start=first, stop=last,
                                      perf_mode=DR)
                                  nc.tensor.matmul(
                                      acc[2 + i2][:],
                                      attp_r[:, :, i2 * 128:(i2 + 1) * 128],
                                      kn_pair, start=first, stop=last,
                                      perf_mode=DR)
                      for i2 in range(2):
                          g = ihalf * 2 + i2   # global i-sub == window index
                          nc.scalar.mul(msgs[:, g * 1024:g * 1024 + 512],
                                        acc[i2][:], SCL_MSG)
                          nc.scalar.mul(msgs[:, g * 1024 + 512:(g + 1) * 1024],
                                        acc[2 + i2][:], SCL_MSG)

                      if stage >= 4:
                          # ------- phase 3 for this i-half's two windows ------
                          for w in (2 * ihalf, 2 * ihalf + 1):
                              for ti in range(QT):
                                  t = w * QT + ti
                                  gt = gath_sb[:, t * 128:(t + 1) * 128]
                                  ps_gp = psC.tile([128, H], F32, tag="gp")
                                  nc.tensor.matmul(
                                      ps_gp[:], gt,
                                      msgs[:, w * 1024:w * 1024 + 512],
                                      start=True, stop=True)
                                  ps_gn = psC.tile([128, H], F32, tag="gn")
                                  nc.tensor.matmul(
                                      ps_gn[:], gt,
                                      msgs[:, w * 1024 + 512:(w + 1) * 1024],
                                      start=True, stop=True)
                                  qt = qres[:, t * H:(t + 1) * H]

                                  # score ops: mostly DVE fused multiply-
                                  # reduce; every 4th routed ACT-copy->Pool
                                  # (Pool cannot read PSUM or run the fused
                                  # op) to keep all three engines busy
                                  def score_dve(ps, sc_t, tag):
                                      snk = strm.tile([128, H], BF16,
                                                      tag=tag, bufs=2)
                                      nc.vector.scalar_tensor_tensor(
                                          snk[:], ps[:], 1.0, qt,
                                          ALU.mult, ALU.mult,
                                          accum_out=sc_t)

                                  def score_3eng(ps, sc_t, tag):
                                      cp = strm.tile([128, H], BF16,
                                                     tag=tag + "c", bufs=2)
                                      nc.scalar.copy(cp[:], ps[:])
                                      pr = strm.tile([128, H], BF16,
                                                     tag=tag + "p", bufs=2)
                                      nc.gpsimd.tensor_mul(pr[:], cp[:], qt)
                                      nc.vector.tensor_reduce(
                                          sc_t, pr[:], mybir.AxisListType.X,
                                          ALU.add)

                                  if ti % 3 == 1:
                                      score_3eng(ps_gp, sc_pos[:, t:t + 1],
                                                 "sp")
                                      score_3eng(ps_gn, sc_neg[:, t:t + 1],
                                                 "sn")
                                  else:
                                      score_dve(ps_gp, sc_pos[:, t:t + 1],
                                                "sink")
                                      score_dve(ps_gn, sc_neg[:, t:t + 1],
                                                "sink2")
                  ps3.__exit__(None, None, None)
                  ps2.__exit__(None, None, None)

              if stage >= 4:
                  # loss = sum(mask * (ln(neg) - ln(pos)))
                  lpos = bandp.tile([128, T], F32, tag="lpos")
                  nc.scalar.activation(lpos[:], sc_pos[:], AF.Ln)
                  lneg = bandp.tile([128, T], F32, tag="lneg")
                  nc.scalar.activation(lneg[:], sc_neg[:], AF.Ln)
                  dl = bandp.tile([128, T], F32, tag="dl")
                  nc.vector.tensor_sub(dl[:], lneg[:], lpos[:])
                  mk = bandp.tile([128, T], F32, tag="mk")
                  nc.sync.dma_start(mk[:], i_mask[:])
                  nc.vector.tensor_mul(dl[:], dl[:], mk[:])
                  lsum = bandp.tile([128, 1], F32, tag="lsum")
                  nc.vector.tensor_reduce(lsum[:], dl[:], mybir.AxisListType.X,
                                          ALU.add)
                  nc.sync.dma_start(o_loss[:], lsum[:])
              elif stage == 1:
                  dbg = strm2.tile([128, 1], F32, tag="dbg")
                  nc.vector.tensor_copy(dbg[:], kpT_band[:, 0:1])
                  nc.sync.dma_start(o_loss[:], dbg[:])
              elif stage == 2:
                  tmpld = strm2.tile([128, 1], F8, tag="tmpld")
                  nc.sync.dma_start(tmpld[:],
                                    agout[NW - 1][NCORES - 1][:, 0:1])
                  dbg = strm2.tile([128, 1], F32, tag="dbg")
                  nc.vector.tensor_copy(dbg[:], tmpld[:])
                  nc.sync.dma_start(o_loss[:], dbg[:])
              elif stage == 3:
                  dbg = strm2.tile([128, 1], F32, tag="dbg")
                  nc.vector.tensor_copy(dbg[:], msgs[:, 0:1])
                  nc.sync.dma_start(o_loss[:], dbg[:])

    nc.compile()
    return nc


_CACHE = {}


def get_compiled(T, QT, bias_nonzero, stage=4, reps=1):
    key = (T, QT, bias_nonzero, stage, reps)
    if key not in _CACHE:
        _CACHE[key] = build_program(T, QT, bias_nonzero, stage, reps)
    return _CACHE[key]


def make_in_maps(per_core):
    return [dict(d) for d in per_core]


def kernel(x, node_to_par, p_weight, A_P, W, b):
    per_core, meta = prep_inputs(x, node_to_par, p_weight, A_P, W, b)
    nc = get_compiled(meta["T"], meta["QT"], meta["bias_nonzero"])
    res = bass_utils.run_bass_kernel_spmd(
        nc, make_in_maps(per_core), core_ids=list(range(NCORES)))
    total = np.float64(0.0)
    for c in range(NCORES):
        total += np.asarray(res.results[c]["loss_part"], np.float64).sum()
    return np.float32(total / meta["N"])

